# revision 21
# baseline (speedup 1.0000x reference)
"""Trainium2 Bass kernel for PhysicsInformedHardProj.

Reference computation (per point p in a 1M-point cloud, fp32):
    h = relu(p @ W1 + b1); h = relu(h @ W2 + b2); h = relu(h @ W3 + b3)
    mu  = relu(h @ Wmu + bmu)           # 4 values
    lam = h @ Wlam + blam               # 3 values
    nrm = sqrt((mu0-mu1)^2 + (mu2-mu3)^2)
    mu *= 1/(nrm+1e-8) if nrm > 1 else 1
    lam /= (||lam||_2 + 1e-8)
Outputs: mu (4, N), lam (3, N).

Strategy: pure data parallel over 8 cores (125k points each). On-chip, the
MLP runs in feature-on-partition layout with two points packed per PE
column (block-diagonal weights, K=M=128), the head projection runs as a
swapped matmul (stationary = activations) that lands points-on-partitions,
and all the per-point norm math runs 128-points-wide on DVE/ACT. Output is
written in a packed on-device layout and unpacked on the host.
"""

import os
import sys
from contextlib import ExitStack

import numpy as np

import concourse.bass as bass
import concourse.tile as tile
from concourse import bacc, mybir
from concourse.bass_utils import run_bass_kernel_spmd

# ---------------------------------------------------------------- constants
N = 1_000_000
M = 8                      # cores
NC = N // M                # 125000 points per core
HALF = 62592               # padded half-shard columns (= 15*4096 + 1152)
NPAD = 2 * HALF            # 125184 padded points per core
CHUNKS = [4096] * 15 + [1152]   # matmul column chunks (2 points per column)
OPW = 14 * (HALF // 128)   # packed output width = 14 * 489 = 6846

FP = mybir.dt.float32
# matmul dtype: float32r streams at full rate (1 col/cyc) for >=256 moving
# columns vs 4 cyc/col for plain float32, and measures bit-identical to
# float32 on TRN2 (probe_f32r.py). Set KERNEL_MM_DTYPE=float32 to disable.
MM_R = os.environ.get("KERNEL_MM_DTYPE", "float32r") == "float32r"
FM = mybir.dt.float32r if MM_R else FP

AF = mybir.ActivationFunctionType
ALU = mybir.AluOpType


def _mm(ap):
    return ap


def _np_dt(dt_):
    return np.float32


# ---------------------------------------------------------------- device body
def body(ctx: ExitStack, tc: tile.TileContext, xp, w1p, w2p, w3p, whp, b123,
         bh, op, chunks=CHUNKS):
    """Emit the kernel. All args (after tc) are DRAM APs.

    xp   (4, sum(chunks)) packed transposed input
    w1p  (4, 128)   blockdiag(W1, W1)
    w2p  (128, 128) blockdiag(W2, W2)
    w3p  (128, 128) blockdiag(W3, W3)
    whp  (128, 14)  blockdiag(Whead_perm, Whead_perm), head col order
                    [mu0, mu2, mu1, mu3, lam0, lam1, lam2]
    b123 (128, 3)   col j = [b_j; b_j]
    bh   (128, 14)  every partition = [bhead_perm, bhead_perm]
    op   (128, 14 * sum(chunks)//128) packed output
    """
    nc = tc.nc

    consts = ctx.enter_context(tc.tile_pool(name="consts", bufs=1))
    xpool = ctx.enter_context(tc.tile_pool(name="xin", bufs=2))
    hpool = ctx.enter_context(tc.tile_pool(name="acts", bufs=2))
    ps = ctx.enter_context(tc.tile_pool(name="ps", bufs=3, space="PSUM"))
    hps = ctx.enter_context(tc.tile_pool(name="hps", bufs=2, space="PSUM"))
    opool = ctx.enter_context(tc.tile_pool(name="outs", bufs=2))
    spool = ctx.enter_context(tc.tile_pool(name="stats", bufs=2))

    # constants, loaded once
    w1t = consts.tile([4, 128], FM)
    w2t = consts.tile([128, 128], FM)
    w3t = consts.tile([128, 128], FM)
    wht = consts.tile([128, 14], FM)
    b123t = consts.tile([128, 3], FP)
    bht = consts.tile([128, 14], FP)
    for t, src in ((w1t, w1p), (w2t, w2p), (w3t, w3p), (wht, whp),
                   (b123t, b123), (bht, bh)):
        nc.sync.dma_start(t[:], src)

    # The fused fp32 matmul (internal weight load) supports only ONE sync
    # wait at codegen. Absorb each weight-DMA wait into a tiny dummy matmul
    # so real matmuls never need two waits.
    warm = ctx.enter_context(tc.tile_pool(name="warm", bufs=1, space="PSUM"))
    d = warm.tile([128, 512], FP, tag="warm")
    for i, wt in enumerate((w1t, w2t, w3t, wht)):
        nc.tensor.matmul(d[:wt.shape[1], 2 * i:2 * i + 2], _mm(wt[:]),
                         _mm(wt[:, 0:2]), start=True, stop=True)

    # bf16 heater matmuls: fp32/fp32r matmuls do not trip the PE HAM
    # activity monitor, so without these the PE clock stays at 1.2 GHz
    # (measured: identical cycle counts, 2x wall time). An initial burst
    # warms it; periodic singles (inserted in the main loop) keep it warm.
    hwa = consts.tile([128, 128], mybir.dt.bfloat16)
    hwb = consts.tile([128, 512], mybir.dt.bfloat16)
    nc.vector.memset(hwa[:], 0.5)
    nc.vector.memset(hwb[:], 0.5)

    # Chain all PE matmuls in emission order (nosync deps): the scheduler
    # otherwise interleaves head matmuls (new stationary every inst) with
    # layer matmuls, forcing a weight reload on every matmul (~750ns vs
    # ~230ns measured back-to-back).
    from concourse.tile import add_dep_helper
    pe_chain = [None]

    def mm(*args, **kw):
        inst = nc.tensor.matmul(*args, **kw)
        raw = inst.ins
        if pe_chain[0] is not None:
            add_dep_helper(raw, pe_chain[0], sync=False,
                           reason="PE weight-group ordering")
        pe_chain[0] = raw
        return inst

    def heater(n=1):
        for _ in range(n):
            mm(d[:, :512], hwa[:], hwb[:], start=True, stop=True)

    heater(24)   # initial ~5us warm-up burst

    cs = 0      # input column offset
    os_ = 0     # packed output column offset
    for C in chunks:
        G = C // 128
        heater(1)
        xt = xpool.tile([4, C], FM)
        nc.sync.dma_start(xt[:], xp[:, cs:cs + C])

        h1 = hpool.tile([128, C], FM, tag="h1")
        h2 = hpool.tile([128, C], FM, tag="h2")
        h3 = hpool.tile([128, C], FM, tag="h3")

        # three MLP layers; relu+bias on ACT (L1, L3) and DVE (L2)
        for c0 in range(0, C, 512):
            w = min(512, C - c0)
            p1 = ps.tile([128, 512], FP, tag="ps")
            mm(p1[:, :w], _mm(w1t[:]), _mm(xt[:, c0:c0 + w]),
               start=True, stop=True)
            nc.scalar.activation(h1[:, c0:c0 + w], p1[:, :w], AF.Relu,
                                 bias=b123t[:, 0:1])
        heater(1)
        for c0 in range(0, C, 512):
            w = min(512, C - c0)
            p2 = ps.tile([128, 512], FP, tag="ps")
            mm(p2[:, :w], _mm(w2t[:]), _mm(h1[:, c0:c0 + w]),
               start=True, stop=True)
            nc.vector.tensor_scalar(h2[:, c0:c0 + w], p2[:, :w],
                                    b123t[:, 1:2], 0.0, ALU.add, ALU.max)
        heater(1)
        for c0 in range(0, C, 512):
            w = min(512, C - c0)
            p3 = ps.tile([128, 512], FP, tag="ps")
            mm(p3[:, :w], _mm(w3t[:]), _mm(h2[:, c0:c0 + w]),
               start=True, stop=True)
            nc.scalar.activation(h3[:, c0:c0 + w], p3[:, :w], AF.Relu,
                                 bias=b123t[:, 2:3])

        # head projection, swapped matmul: out partitions = points
        # (512-wide tile keeps each slot bank-aligned in PSUM)
        hp = hps.tile([128, 512], FP, tag="hp" if C == 4096 else "hp_tail")
        heater(1)
        for g in range(G):
            mm(hp[:, 14 * g:14 * g + 14],
               _mm(h3[:, 128 * g:128 * g + 128]), _mm(wht[:]),
               start=True, stop=True)

        # per-point stats, 128 points per partition-row group
        Hb = opool.tile([128, 14 * G], FP, tag="Hb")
        hp4 = hp[:, :14 * G].rearrange("p (g h e) -> p g h e", h=2, e=7)
        Hb4 = Hb[:].rearrange("p (g h e) -> p g h e", h=2, e=7)
        bh_b = bass.AP(tensor=bht[:].tensor, offset=bht[:].offset,
                       ap=[list(bht[:].ap[0]), [0, G], [7, 2], [1, 7]])
        nc.vector.tensor_add(Hb4, hp4, bh_b)

        mu4 = Hb4[:, :, :, 0:4]
        lam4 = Hb4[:, :, :, 4:7]
        nc.vector.tensor_scalar_max(mu4, mu4, 0.0)   # relu(mu)

        D = spool.tile([128, G, 2, 2], FP, tag="D")
        nc.vector.tensor_sub(D[:], Hb4[:, :, :, 0:2], Hb4[:, :, :, 2:4])
        nc.vector.tensor_mul(D[:], D[:], D[:])
        NSQ = spool.tile([128, G, 2], FP, tag="NSQ")
        nc.vector.reduce_sum(NSQ[:], D[:], axis=mybir.AxisListType.X)
        NRM = spool.tile([128, G, 2], FP, tag="NRM")
        nc.scalar.activation(NRM[:], NSQ[:], AF.Sqrt)
        nc.vector.tensor_scalar_max(NRM[:], NRM[:], 1.0)
        SMU = spool.tile([128, G, 2], FP, tag="SMU")
        nc.vector.reciprocal(SMU[:], NRM[:])

        L3q = spool.tile([128, G, 2, 3], FP, tag="L3q")
        nc.vector.tensor_mul(L3q[:], lam4, lam4)
        LSQ = spool.tile([128, G, 2], FP, tag="LSQ")
        nc.vector.reduce_sum(LSQ[:], L3q[:], axis=mybir.AxisListType.X)
        LNR = spool.tile([128, G, 2], FP, tag="LNR")
        nc.scalar.activation(LNR[:], LSQ[:], AF.Sqrt)
        nc.vector.tensor_scalar_add(LNR[:], LNR[:], 1e-8)
        SLAM = spool.tile([128, G, 2], FP, tag="SLAM")
        nc.vector.reciprocal(SLAM[:], LNR[:])

        nc.vector.tensor_mul(mu4, mu4, SMU[:].to_broadcast((128, G, 2, 4)))
        nc.vector.tensor_mul(lam4, lam4, SLAM[:].to_broadcast((128, G, 2, 3)))

        nc.sync.dma_start(op[:, os_:os_ + 14 * G], Hb[:])

        cs += C
        os_ += 14 * G


# ---------------------------------------------------------------- host glue
def _prep_consts(W1, b1, W2, b2, W3, b3, Wmu, bmu, Wlam, blam):
    f = np.float32
    w1p = np.zeros((4, 128), f)
    w1p[0:2, 0:64] = W1
    w1p[2:4, 64:128] = W1
    w2p = np.zeros((128, 128), f)
    w2p[0:64, 0:64] = W2
    w2p[64:128, 64:128] = W2
    w3p = np.zeros((128, 128), f)
    w3p[0:64, 0:64] = W3
    w3p[64:128, 64:128] = W3
    perm = [0, 2, 1, 3]
    whperm = np.concatenate([Wmu[:, perm], Wlam], axis=1).astype(f)  # (64,7)
    whp = np.zeros((128, 14), f)
    whp[0:64, 0:7] = whperm
    whp[64:128, 7:14] = whperm
    b123 = np.stack([np.concatenate([b1, b1]), np.concatenate([b2, b2]),
                     np.concatenate([b3, b3])], axis=1).astype(f)    # (128,3)
    bhperm = np.concatenate([np.asarray(bmu)[perm], blam]).astype(f)  # (7,)
    bh = np.tile(np.concatenate([bhperm, bhperm])[None, :], (128, 1)).astype(f)
    return dict(w1p=w1p, w2p=w2p, w3p=w3p, whp=whp, b123=b123, bh=bh)


def _pack_input(shard):
    """(NC, 2) f32 -> (4, HALF) packed transposed."""
    pad = np.zeros((NPAD, 2), np.float32)
    pad[:shard.shape[0]] = shard
    xt = pad.T  # (2, NPAD)
    return np.ascontiguousarray(
        np.concatenate([xt[:, :HALF], xt[:, HALF:]], axis=0))


def _unpack_output(op):
    """(128, OPW) packed -> mu (4, NC), lam (3, NC)."""
    mu = np.empty((4, NPAD), np.float32)
    lam = np.empty((3, NPAD), np.float32)
    inv = [0, 2, 1, 3]  # packed col order [mu0, mu2, mu1, mu3]: row e at col inv[e]
    cs = 0
    os_ = 0
    for C in CHUNKS:
        G = C // 128
        blk = op[:, os_:os_ + 14 * G].reshape(128, G, 2, 7)
        v = blk.transpose(3, 2, 1, 0).reshape(7, 2, G * 128)
        for h in range(2):
            dst = slice(h * HALF + cs, h * HALF + cs + C)
            mu[:, dst] = v[inv, h]
            lam[:, dst] = v[4:7, h]
        cs += C
        os_ += 14 * G
    return mu[:, :NC], lam[:, :NC]


_CACHED_NC = None


def _build_nc():
    global _CACHED_NC
    if _CACHED_NC is not None:
        return _CACHED_NC
    nc = bacc.Bacc("TRN2", target_bir_lowering=False, debug=False,
                   enable_asserts=False, num_devices=M)
    xp = nc.dram_tensor("xp", [4, HALF], FM, kind="ExternalInput").ap()
    w1p = nc.dram_tensor("w1p", [4, 128], FM, kind="ExternalInput").ap()
    w2p = nc.dram_tensor("w2p", [128, 128], FM, kind="ExternalInput").ap()
    w3p = nc.dram_tensor("w3p", [128, 128], FM, kind="ExternalInput").ap()
    whp = nc.dram_tensor("whp", [128, 14], FM, kind="ExternalInput").ap()
    b123 = nc.dram_tensor("b123", [128, 3], FP, kind="ExternalInput").ap()
    bh = nc.dram_tensor("bh", [128, 14], FP, kind="ExternalInput").ap()
    op = nc.dram_tensor("op", [128, OPW], FP, kind="ExternalOutput").ap()
    with tile.TileContext(nc) as tc:
        with ExitStack() as ctx:
            body(ctx, tc, xp, w1p, w2p, w3p, whp, b123, bh, op)
    nc.compile()
    _CACHED_NC = nc
    return nc


def kernel(point_cloud, W1, b1, W2, b2, W3, b3, Wmu, bmu, Wlam, blam,
           _want_results=False, **run_kwargs):
    consts = _prep_consts(W1, b1, W2, b2, W3, b3, Wmu, bmu, Wlam, blam)
    consts = {k: np.ascontiguousarray(v, np.float32)
              for k, v in consts.items()}
    pc = np.asarray(point_cloud, np.float32)
    in_maps = []
    for c in range(M):
        m = dict(consts)
        m["xp"] = _pack_input(pc[c * NC:(c + 1) * NC])
        in_maps.append(m)
    nc = _build_nc()
    res = run_bass_kernel_spmd(nc, in_maps, core_ids=list(range(M)),
                               **run_kwargs)
    mu = np.empty((4, N), np.float32)
    lam = np.empty((3, N), np.float32)
    for c in range(M):
        mu_c, lam_c = _unpack_output(res.results[c]["op"])
        mu[:, c * NC:(c + 1) * NC] = mu_c
        lam[:, c * NC:(c + 1) * NC] = lam_c
    if _want_results:
        return (mu, lam), res
    return mu, lam


# revision 22
# speedup vs baseline: 1.2353x; 1.2353x over previous
"""Trainium2 Bass kernel for PhysicsInformedHardProj.

Reference computation (per point p in a 1M-point cloud, fp32):
    h = relu(p @ W1 + b1); h = relu(h @ W2 + b2); h = relu(h @ W3 + b3)
    mu  = relu(h @ Wmu + bmu)           # 4 values
    lam = h @ Wlam + blam               # 3 values
    nrm = sqrt((mu0-mu1)^2 + (mu2-mu3)^2)
    mu *= 1/(nrm+1e-8) if nrm > 1 else 1
    lam /= (||lam||_2 + 1e-8)
Outputs: mu (4, N), lam (3, N).

Strategy: pure data parallel over 8 cores (125k points each). On-chip, the
MLP runs in feature-on-partition layout with two points packed per PE
column (block-diagonal weights, K=M=128), the head projection runs as a
swapped matmul (stationary = activations) that lands points-on-partitions,
and all the per-point norm math runs 128-points-wide on DVE/ACT. Output is
written in a packed on-device layout and unpacked on the host.
"""

import os
import sys
from contextlib import ExitStack

import numpy as np

import concourse.bass as bass
import concourse.tile as tile
from concourse import bacc, mybir
from concourse.bass_utils import run_bass_kernel_spmd

# ---------------------------------------------------------------- constants
N = 1_000_000
M = 8                      # cores
NC = N // M                # 125000 points per core
HALF = 62592               # padded half-shard columns (= 15*4096 + 1152)
NPAD = 2 * HALF            # 125184 padded points per core
CHUNKS = [4096] * 15 + [1152]   # matmul column chunks (2 points per column)
OPW = 14 * (HALF // 128)   # packed output width = 14 * 489 = 6846

FP = mybir.dt.float32
# matmul dtype: float32r streams at full rate (1 col/cyc) for >=256 moving
# columns vs 4 cyc/col for plain float32, and measures bit-identical to
# float32 on TRN2 (probe_f32r.py). Set KERNEL_MM_DTYPE=float32 to disable.
MM_R = os.environ.get("KERNEL_MM_DTYPE", "float32r") == "float32r"
FM = mybir.dt.float32r if MM_R else FP

AF = mybir.ActivationFunctionType
ALU = mybir.AluOpType


def _mm(ap):
    return ap


def _np_dt(dt_):
    return np.float32


# ---------------------------------------------------------------- device body
def body(ctx: ExitStack, tc: tile.TileContext, xp, w1p, w2p, w3p, whp, b123,
         bh, op, chunks=CHUNKS):
    """Emit the kernel. All args (after tc) are DRAM APs.

    xp   (4, sum(chunks)) packed transposed input
    w1p  (4, 128)   blockdiag(W1, W1)
    w2p  (128, 128) blockdiag(W2, W2)
    w3p  (128, 128) blockdiag(W3, W3)
    whp  (128, 14)  blockdiag(Whead_perm, Whead_perm), head col order
                    [mu0, mu2, mu1, mu3, lam0, lam1, lam2]
    b123 (128, 3)   col j = [b_j; b_j]
    bh   (128, 14)  every partition = [bhead_perm, bhead_perm]
    op   (128, 14 * sum(chunks)//128) packed output
    """
    nc = tc.nc

    consts = ctx.enter_context(tc.tile_pool(name="consts", bufs=1))
    xpool = ctx.enter_context(tc.tile_pool(name="xin", bufs=2))
    hpool = ctx.enter_context(tc.tile_pool(name="acts", bufs=2))
    ps = ctx.enter_context(tc.tile_pool(name="ps", bufs=5, space="PSUM"))
    hps = ctx.enter_context(tc.tile_pool(name="hps", bufs=2, space="PSUM"))
    opool = ctx.enter_context(tc.tile_pool(name="outs", bufs=2))
    spool = ctx.enter_context(tc.tile_pool(name="stats", bufs=2))

    # constants, loaded once
    w1t = consts.tile([4, 128], FM)
    w2t = consts.tile([128, 128], FM)
    w3t = consts.tile([128, 128], FM)
    wht = consts.tile([128, 14], FM)
    b123t = consts.tile([128, 3], FP)
    bht = consts.tile([128, 14], FP)
    for t, src in ((w1t, w1p), (w2t, w2p), (w3t, w3p), (wht, whp),
                   (b123t, b123), (bht, bh)):
        nc.sync.dma_start(t[:], src)

    # The fused fp32 matmul (internal weight load) supports only ONE sync
    # wait at codegen. Absorb each weight-DMA wait into a tiny dummy matmul
    # so real matmuls never need two waits.
    warm = ctx.enter_context(tc.tile_pool(name="warm", bufs=1, space="PSUM"))
    d = warm.tile([128, 512], FP, tag="warm")
    for i, wt in enumerate((w1t, w2t, w3t, wht)):
        nc.tensor.matmul(d[:wt.shape[1], 2 * i:2 * i + 2], _mm(wt[:]),
                         _mm(wt[:, 0:2]), start=True, stop=True)

    # bf16 heater matmuls: fp32/fp32r matmuls do not trip the PE HAM
    # activity monitor, so without these the PE clock stays at 1.2 GHz
    # (measured: identical cycle counts, 2x wall time). An initial burst
    # warms it; periodic singles (inserted in the main loop) keep it warm.
    hwa = consts.tile([128, 128], mybir.dt.bfloat16)
    hwb = consts.tile([128, 512], mybir.dt.bfloat16)
    nc.vector.memset(hwa[:], 0.5)
    nc.vector.memset(hwb[:], 0.5)

    # Chain all PE matmuls in emission order (nosync deps): the scheduler
    # otherwise interleaves head matmuls (new stationary every inst) with
    # layer matmuls, forcing a weight reload on every matmul (~750ns vs
    # ~230ns measured back-to-back).
    from concourse.tile import add_dep_helper
    pe_chain = [None]

    def mm(*args, **kw):
        inst = nc.tensor.matmul(*args, **kw)
        raw = inst.ins
        if pe_chain[0] is not None:
            add_dep_helper(raw, pe_chain[0], sync=False,
                           reason="PE weight-group ordering")
        pe_chain[0] = raw
        return inst

    def heater(n=1):
        for _ in range(n):
            mm(d[:, :512], hwa[:], hwb[:], start=True, stop=True)

    heater(24)   # initial ~5us warm-up burst

    cs = 0      # input column offset
    os_ = 0     # packed output column offset
    for C in chunks:
        G = C // 128
        heater(1)
        xt = xpool.tile([4, C], FM)
        nc.sync.dma_start(xt[:], xp[:, cs:cs + C])

        h1 = hpool.tile([128, C], FM, tag="h1")
        h2 = hpool.tile([128, C], FM, tag="h2")
        h3 = hpool.tile([128, C], FM, tag="h3")

        # three MLP layers; relu+bias on ACT (L1, L3) and DVE (L2)
        for c0 in range(0, C, 512):
            w = min(512, C - c0)
            p1 = ps.tile([128, 512], FP, tag="ps")
            mm(p1[:, :w], _mm(w1t[:]), _mm(xt[:, c0:c0 + w]),
               start=True, stop=True)
            nc.scalar.activation(h1[:, c0:c0 + w], p1[:, :w], AF.Relu,
                                 bias=b123t[:, 0:1])
        heater(1)
        for c0 in range(0, C, 512):
            w = min(512, C - c0)
            p2 = ps.tile([128, 512], FP, tag="ps")
            mm(p2[:, :w], _mm(w2t[:]), _mm(h1[:, c0:c0 + w]),
               start=True, stop=True)
            nc.vector.tensor_scalar(h2[:, c0:c0 + w], p2[:, :w],
                                    b123t[:, 1:2], 0.0, ALU.add, ALU.max)
        heater(1)
        for k, c0 in enumerate(range(0, C, 512)):
            w = min(512, C - c0)
            p3 = ps.tile([128, 512], FP, tag="ps")
            mm(p3[:, :w], _mm(w3t[:]), _mm(h2[:, c0:c0 + w]),
               start=True, stop=True)
            if k % 4 == 3:   # spread L3 evac: 1/4 on DVE, 3/4 on ACT
                nc.vector.tensor_scalar(h3[:, c0:c0 + w], p3[:, :w],
                                        b123t[:, 2:3], 0.0, ALU.add, ALU.max)
            else:
                nc.scalar.activation(h3[:, c0:c0 + w], p3[:, :w], AF.Relu,
                                     bias=b123t[:, 2:3])

        # head projection, swapped matmul: out partitions = points
        # (512-wide tile keeps each slot bank-aligned in PSUM)
        hp = hps.tile([128, 512], FP, tag="hp")
        heater(1)
        for g in range(G):
            mm(hp[:, 14 * g:14 * g + 14],
               _mm(h3[:, 128 * g:128 * g + 128]), _mm(wht[:]),
               start=True, stop=True)

        # per-point stats, 128 points per partition-row group
        Hb = opool.tile([128, 14 * G], FP, tag="Hb")
        hp4 = hp[:, :14 * G].rearrange("p (g h e) -> p g h e", h=2, e=7)
        Hb4 = Hb[:].rearrange("p (g h e) -> p g h e", h=2, e=7)
        bh_b = bass.AP(tensor=bht[:].tensor, offset=bht[:].offset,
                       ap=[list(bht[:].ap[0]), [0, G], [7, 2], [1, 7]])
        nc.vector.tensor_add(Hb4, hp4, bh_b)

        mu4 = Hb4[:, :, :, 0:4]
        lam4 = Hb4[:, :, :, 4:7]
        nc.vector.tensor_scalar_max(mu4, mu4, 0.0)   # relu(mu)

        D = spool.tile([128, G, 2, 2], FP, tag="D")
        nc.vector.tensor_sub(D[:], Hb4[:, :, :, 0:2], Hb4[:, :, :, 2:4])
        nc.vector.tensor_mul(D[:], D[:], D[:])
        NSQ = spool.tile([128, G, 2], FP, tag="NSQ")
        nc.vector.reduce_sum(NSQ[:], D[:], axis=mybir.AxisListType.X)
        NRM = spool.tile([128, G, 2], FP, tag="NRM")
        nc.scalar.activation(NRM[:], NSQ[:], AF.Sqrt)
        nc.vector.tensor_scalar_max(NRM[:], NRM[:], 1.0)
        SMU = spool.tile([128, G, 2], FP, tag="SMU")
        nc.vector.reciprocal(SMU[:], NRM[:])

        L3q = spool.tile([128, G, 2, 3], FP, tag="L3q")
        nc.vector.tensor_mul(L3q[:], lam4, lam4)
        LSQ = spool.tile([128, G, 2], FP, tag="LSQ")
        nc.vector.reduce_sum(LSQ[:], L3q[:], axis=mybir.AxisListType.X)
        LNR = spool.tile([128, G, 2], FP, tag="LNR")
        nc.scalar.activation(LNR[:], LSQ[:], AF.Sqrt)
        nc.vector.tensor_scalar_add(LNR[:], LNR[:], 1e-8)
        SLAM = spool.tile([128, G, 2], FP, tag="SLAM")
        nc.vector.reciprocal(SLAM[:], LNR[:])

        nc.vector.tensor_mul(mu4, mu4, SMU[:].to_broadcast((128, G, 2, 4)))
        nc.vector.tensor_mul(lam4, lam4, SLAM[:].to_broadcast((128, G, 2, 3)))

        nc.sync.dma_start(op[:, os_:os_ + 14 * G], Hb[:])

        cs += C
        os_ += 14 * G


# ---------------------------------------------------------------- host glue
def _prep_consts(W1, b1, W2, b2, W3, b3, Wmu, bmu, Wlam, blam):
    f = np.float32
    w1p = np.zeros((4, 128), f)
    w1p[0:2, 0:64] = W1
    w1p[2:4, 64:128] = W1
    w2p = np.zeros((128, 128), f)
    w2p[0:64, 0:64] = W2
    w2p[64:128, 64:128] = W2
    w3p = np.zeros((128, 128), f)
    w3p[0:64, 0:64] = W3
    w3p[64:128, 64:128] = W3
    perm = [0, 2, 1, 3]
    whperm = np.concatenate([Wmu[:, perm], Wlam], axis=1).astype(f)  # (64,7)
    whp = np.zeros((128, 14), f)
    whp[0:64, 0:7] = whperm
    whp[64:128, 7:14] = whperm
    b123 = np.stack([np.concatenate([b1, b1]), np.concatenate([b2, b2]),
                     np.concatenate([b3, b3])], axis=1).astype(f)    # (128,3)
    bhperm = np.concatenate([np.asarray(bmu)[perm], blam]).astype(f)  # (7,)
    bh = np.tile(np.concatenate([bhperm, bhperm])[None, :], (128, 1)).astype(f)
    return dict(w1p=w1p, w2p=w2p, w3p=w3p, whp=whp, b123=b123, bh=bh)


def _pack_input(shard):
    """(NC, 2) f32 -> (4, HALF) packed transposed."""
    pad = np.zeros((NPAD, 2), np.float32)
    pad[:shard.shape[0]] = shard
    xt = pad.T  # (2, NPAD)
    return np.ascontiguousarray(
        np.concatenate([xt[:, :HALF], xt[:, HALF:]], axis=0))


def _unpack_output(op):
    """(128, OPW) packed -> mu (4, NC), lam (3, NC)."""
    mu = np.empty((4, NPAD), np.float32)
    lam = np.empty((3, NPAD), np.float32)
    inv = [0, 2, 1, 3]  # packed col order [mu0, mu2, mu1, mu3]: row e at col inv[e]
    cs = 0
    os_ = 0
    for C in CHUNKS:
        G = C // 128
        blk = op[:, os_:os_ + 14 * G].reshape(128, G, 2, 7)
        v = blk.transpose(3, 2, 1, 0).reshape(7, 2, G * 128)
        for h in range(2):
            dst = slice(h * HALF + cs, h * HALF + cs + C)
            mu[:, dst] = v[inv, h]
            lam[:, dst] = v[4:7, h]
        cs += C
        os_ += 14 * G
    return mu[:, :NC], lam[:, :NC]


_CACHED_NC = None


def _build_nc():
    global _CACHED_NC
    if _CACHED_NC is not None:
        return _CACHED_NC
    nc = bacc.Bacc("TRN2", target_bir_lowering=False, debug=False,
                   enable_asserts=False, num_devices=M)
    xp = nc.dram_tensor("xp", [4, HALF], FM, kind="ExternalInput").ap()
    w1p = nc.dram_tensor("w1p", [4, 128], FM, kind="ExternalInput").ap()
    w2p = nc.dram_tensor("w2p", [128, 128], FM, kind="ExternalInput").ap()
    w3p = nc.dram_tensor("w3p", [128, 128], FM, kind="ExternalInput").ap()
    whp = nc.dram_tensor("whp", [128, 14], FM, kind="ExternalInput").ap()
    b123 = nc.dram_tensor("b123", [128, 3], FP, kind="ExternalInput").ap()
    bh = nc.dram_tensor("bh", [128, 14], FP, kind="ExternalInput").ap()
    op = nc.dram_tensor("op", [128, OPW], FP, kind="ExternalOutput").ap()
    with tile.TileContext(nc) as tc:
        with ExitStack() as ctx:
            body(ctx, tc, xp, w1p, w2p, w3p, whp, b123, bh, op)
    nc.compile()
    _CACHED_NC = nc
    return nc


def kernel(point_cloud, W1, b1, W2, b2, W3, b3, Wmu, bmu, Wlam, blam,
           _want_results=False, **run_kwargs):
    consts = _prep_consts(W1, b1, W2, b2, W3, b3, Wmu, bmu, Wlam, blam)
    consts = {k: np.ascontiguousarray(v, np.float32)
              for k, v in consts.items()}
    pc = np.asarray(point_cloud, np.float32)
    in_maps = []
    for c in range(M):
        m = dict(consts)
        m["xp"] = _pack_input(pc[c * NC:(c + 1) * NC])
        in_maps.append(m)
    nc = _build_nc()
    res = run_bass_kernel_spmd(nc, in_maps, core_ids=list(range(M)),
                               **run_kwargs)
    mu = np.empty((4, N), np.float32)
    lam = np.empty((3, N), np.float32)
    for c in range(M):
        mu_c, lam_c = _unpack_output(res.results[c]["op"])
        mu[:, c * NC:(c + 1) * NC] = mu_c
        lam[:, c * NC:(c + 1) * NC] = lam_c
    if _want_results:
        return (mu, lam), res
    return mu, lam


# revision 23
# speedup vs baseline: 1.2761x; 1.0331x over previous
"""Trainium2 Bass kernel for PhysicsInformedHardProj.

Reference computation (per point p in a 1M-point cloud, fp32):
    h = relu(p @ W1 + b1); h = relu(h @ W2 + b2); h = relu(h @ W3 + b3)
    mu  = relu(h @ Wmu + bmu)           # 4 values
    lam = h @ Wlam + blam               # 3 values
    nrm = sqrt((mu0-mu1)^2 + (mu2-mu3)^2)
    mu *= 1/(nrm+1e-8) if nrm > 1 else 1
    lam /= (||lam||_2 + 1e-8)
Outputs: mu (4, N), lam (3, N).

Strategy: pure data parallel over 8 cores (125k points each). On-chip, the
MLP runs in feature-on-partition layout with two points packed per PE
column (block-diagonal weights, K=M=128), the head projection runs as a
swapped matmul (stationary = activations) that lands points-on-partitions,
and all the per-point norm math runs 128-points-wide on DVE/ACT. Output is
written in a packed on-device layout and unpacked on the host.
"""

import os
import sys
from contextlib import ExitStack

import numpy as np

import concourse.bass as bass
import concourse.tile as tile
from concourse import bacc, mybir
from concourse.bass_utils import run_bass_kernel_spmd

# ---------------------------------------------------------------- constants
N = 1_000_000
M = 8                      # cores
NC = N // M                # 125000 points per core
HALF = 62592               # padded half-shard columns (= 15*4096 + 1152)
NPAD = 2 * HALF            # 125184 padded points per core
CHUNKS = [4096] * 15 + [1152]   # matmul column chunks (2 points per column)
OPW = 14 * (HALF // 128)   # packed output width = 14 * 489 = 6846

FP = mybir.dt.float32
# matmul dtype: float32r streams at full rate (1 col/cyc) for >=256 moving
# columns vs 4 cyc/col for plain float32, and measures bit-identical to
# float32 on TRN2 (probe_f32r.py). Set KERNEL_MM_DTYPE=float32 to disable.
MM_R = os.environ.get("KERNEL_MM_DTYPE", "float32r") == "float32r"
FM = mybir.dt.float32r if MM_R else FP

AF = mybir.ActivationFunctionType
ALU = mybir.AluOpType


def _mm(ap):
    return ap


def _np_dt(dt_):
    return np.float32


# ---------------------------------------------------------------- device body
def body(ctx: ExitStack, tc: tile.TileContext, xp, w1p, w2p, w3p, whp, b123,
         bh, op, chunks=CHUNKS):
    """Emit the kernel. All args (after tc) are DRAM APs.

    xp   (4, sum(chunks)) packed transposed input
    w1p  (4, 128)   blockdiag(W1, W1)
    w2p  (128, 128) blockdiag(W2, W2)
    w3p  (128, 128) blockdiag(W3, W3)
    whp  (128, 14)  blockdiag(Whead_perm, Whead_perm), head col order
                    [mu0, mu2, mu1, mu3, lam0, lam1, lam2]
    b123 (128, 3)   col j = [b_j; b_j]
    bh   (128, 14)  every partition = [bhead_perm, bhead_perm]
    op   (128, 14 * sum(chunks)//128) packed output
    """
    nc = tc.nc

    consts = ctx.enter_context(tc.tile_pool(name="consts", bufs=1))
    xpool = ctx.enter_context(tc.tile_pool(name="xin", bufs=2))
    hpool = ctx.enter_context(tc.tile_pool(name="acts", bufs=2))
    ps = ctx.enter_context(tc.tile_pool(name="ps", bufs=5, space="PSUM"))
    hps = ctx.enter_context(tc.tile_pool(name="hps", bufs=2, space="PSUM"))
    opool = ctx.enter_context(tc.tile_pool(name="outs", bufs=2))
    spool = ctx.enter_context(tc.tile_pool(name="stats", bufs=2))

    # constants, loaded once
    w1t = consts.tile([4, 128], FM)
    w2t = consts.tile([128, 128], FM)
    w3t = consts.tile([128, 128], FM)
    wht = consts.tile([128, 14], FM)
    b123t = consts.tile([128, 3], FP)
    bht = consts.tile([128, 14], FP)
    for t, src in ((w1t, w1p), (w2t, w2p), (w3t, w3p), (wht, whp),
                   (b123t, b123), (bht, bh)):
        nc.sync.dma_start(t[:], src)

    # The fused fp32 matmul (internal weight load) supports only ONE sync
    # wait at codegen. Absorb each weight-DMA wait into a tiny dummy matmul
    # so real matmuls never need two waits.
    warm = ctx.enter_context(tc.tile_pool(name="warm", bufs=1, space="PSUM"))
    d = warm.tile([128, 512], FP, tag="warm")
    for i, wt in enumerate((w1t, w2t, w3t, wht)):
        nc.tensor.matmul(d[:wt.shape[1], 2 * i:2 * i + 2], _mm(wt[:]),
                         _mm(wt[:, 0:2]), start=True, stop=True)

    # bf16 heater matmuls: fp32/fp32r matmuls do not trip the PE HAM
    # activity monitor, so without these the PE clock stays at 1.2 GHz
    # (measured: identical cycle counts, 2x wall time). An initial burst
    # warms it; periodic singles (inserted in the main loop) keep it warm.
    hwa = consts.tile([128, 128], mybir.dt.bfloat16)
    hwb = consts.tile([128, 512], mybir.dt.bfloat16)
    nc.vector.memset(hwa[:], 0.5)
    nc.vector.memset(hwb[:], 0.5)

    # Chain all PE matmuls in emission order (nosync deps): the scheduler
    # otherwise interleaves head matmuls (new stationary every inst) with
    # layer matmuls, forcing a weight reload on every matmul (~750ns vs
    # ~230ns measured back-to-back).
    from concourse.tile import add_dep_helper
    pe_chain = [None]

    def mm(*args, **kw):
        inst = nc.tensor.matmul(*args, **kw)
        raw = inst.ins
        if pe_chain[0] is not None:
            add_dep_helper(raw, pe_chain[0], sync=False,
                           reason="PE weight-group ordering")
        pe_chain[0] = raw
        return inst

    def heater(n=1):
        for _ in range(n):
            mm(d[:, :512], hwa[:], hwb[:], start=True, stop=True)

    heater(24)   # initial ~5us warm-up burst

    cs = 0      # input column offset
    os_ = 0     # packed output column offset
    for C in chunks:
        G = C // 128
        heater(1)
        xt = xpool.tile([4, C], FM)
        nc.sync.dma_start(xt[:], xp[:, cs:cs + C])

        h1 = hpool.tile([128, C], FM, tag="h1")
        h2 = hpool.tile([128, C], FM, tag="h2")
        h3 = hpool.tile([128, C], FM, tag="h3")

        # three MLP layers; relu+bias on ACT (L1, L3) and DVE (L2)
        for c0 in range(0, C, 512):
            w = min(512, C - c0)
            p1 = ps.tile([128, 512], FP, tag="ps")
            mm(p1[:, :w], _mm(w1t[:]), _mm(xt[:, c0:c0 + w]),
               start=True, stop=True)
            nc.scalar.activation(h1[:, c0:c0 + w], p1[:, :w], AF.Relu,
                                 bias=b123t[:, 0:1])
        heater(1)
        for c0 in range(0, C, 512):
            w = min(512, C - c0)
            p2 = ps.tile([128, 512], FP, tag="ps")
            mm(p2[:, :w], _mm(w2t[:]), _mm(h1[:, c0:c0 + w]),
               start=True, stop=True)
            nc.vector.tensor_scalar(h2[:, c0:c0 + w], p2[:, :w],
                                    b123t[:, 1:2], 0.0, ALU.add, ALU.max)
        heater(1)
        for k, c0 in enumerate(range(0, C, 512)):
            w = min(512, C - c0)
            p3 = ps.tile([128, 512], FP, tag="ps")
            mm(p3[:, :w], _mm(w3t[:]), _mm(h2[:, c0:c0 + w]),
               start=True, stop=True)
            if k % 8 == 7:   # spread L3 evac: 1/8 on DVE, 7/8 on ACT
                nc.vector.tensor_scalar(h3[:, c0:c0 + w], p3[:, :w],
                                        b123t[:, 2:3], 0.0, ALU.add, ALU.max)
            else:
                nc.scalar.activation(h3[:, c0:c0 + w], p3[:, :w], AF.Relu,
                                     bias=b123t[:, 2:3])

        # head projection, swapped matmul: out partitions = points
        # (512-wide tile keeps each slot bank-aligned in PSUM)
        hp = hps.tile([128, 512], FP, tag="hp")
        heater(1)
        for g in range(G):
            mm(hp[:, 14 * g:14 * g + 14],
               _mm(h3[:, 128 * g:128 * g + 128]), _mm(wht[:]),
               start=True, stop=True)

        # per-point stats, 128 points per partition-row group
        Hb = opool.tile([128, 14 * G], FP, tag="Hb")
        hp4 = hp[:, :14 * G].rearrange("p (g h e) -> p g h e", h=2, e=7)
        Hb4 = Hb[:].rearrange("p (g h e) -> p g h e", h=2, e=7)
        bh_b = bass.AP(tensor=bht[:].tensor, offset=bht[:].offset,
                       ap=[list(bht[:].ap[0]), [0, G], [7, 2], [1, 7]])
        nc.vector.tensor_add(Hb4, hp4, bh_b)

        mu4 = Hb4[:, :, :, 0:4]
        lam4 = Hb4[:, :, :, 4:7]
        nc.vector.tensor_scalar_max(mu4, mu4, 0.0)   # relu(mu)

        D = spool.tile([128, G, 2, 2], FP, tag="D")
        nc.gpsimd.tensor_sub(D[:], Hb4[:, :, :, 0:2], Hb4[:, :, :, 2:4])
        nc.gpsimd.tensor_mul(D[:], D[:], D[:])
        NSQ = spool.tile([128, G, 2], FP, tag="NSQ")
        nc.vector.reduce_sum(NSQ[:], D[:], axis=mybir.AxisListType.X)
        NRM = spool.tile([128, G, 2], FP, tag="NRM")
        nc.scalar.activation(NRM[:], NSQ[:], AF.Sqrt)
        nc.vector.tensor_scalar_max(NRM[:], NRM[:], 1.0)
        SMU = spool.tile([128, G, 2], FP, tag="SMU")
        nc.vector.reciprocal(SMU[:], NRM[:])

        L3q = spool.tile([128, G, 2, 3], FP, tag="L3q")
        nc.gpsimd.tensor_mul(L3q[:], lam4, lam4)
        LSQ = spool.tile([128, G, 2], FP, tag="LSQ")
        nc.vector.reduce_sum(LSQ[:], L3q[:], axis=mybir.AxisListType.X)
        LNR = spool.tile([128, G, 2], FP, tag="LNR")
        nc.scalar.activation(LNR[:], LSQ[:], AF.Sqrt)
        nc.gpsimd.tensor_scalar_add(LNR[:], LNR[:], 1e-8)
        SLAM = spool.tile([128, G, 2], FP, tag="SLAM")
        nc.vector.reciprocal(SLAM[:], LNR[:])

        nc.gpsimd.tensor_mul(mu4, mu4, SMU[:].to_broadcast((128, G, 2, 4)))
        nc.gpsimd.tensor_mul(lam4, lam4,
                             SLAM[:].to_broadcast((128, G, 2, 3)))

        nc.sync.dma_start(op[:, os_:os_ + 14 * G], Hb[:])

        cs += C
        os_ += 14 * G


# ---------------------------------------------------------------- host glue
def _prep_consts(W1, b1, W2, b2, W3, b3, Wmu, bmu, Wlam, blam):
    f = np.float32
    w1p = np.zeros((4, 128), f)
    w1p[0:2, 0:64] = W1
    w1p[2:4, 64:128] = W1
    w2p = np.zeros((128, 128), f)
    w2p[0:64, 0:64] = W2
    w2p[64:128, 64:128] = W2
    w3p = np.zeros((128, 128), f)
    w3p[0:64, 0:64] = W3
    w3p[64:128, 64:128] = W3
    perm = [0, 2, 1, 3]
    whperm = np.concatenate([Wmu[:, perm], Wlam], axis=1).astype(f)  # (64,7)
    whp = np.zeros((128, 14), f)
    whp[0:64, 0:7] = whperm
    whp[64:128, 7:14] = whperm
    b123 = np.stack([np.concatenate([b1, b1]), np.concatenate([b2, b2]),
                     np.concatenate([b3, b3])], axis=1).astype(f)    # (128,3)
    bhperm = np.concatenate([np.asarray(bmu)[perm], blam]).astype(f)  # (7,)
    bh = np.tile(np.concatenate([bhperm, bhperm])[None, :], (128, 1)).astype(f)
    return dict(w1p=w1p, w2p=w2p, w3p=w3p, whp=whp, b123=b123, bh=bh)


def _pack_input(shard):
    """(NC, 2) f32 -> (4, HALF) packed transposed."""
    pad = np.zeros((NPAD, 2), np.float32)
    pad[:shard.shape[0]] = shard
    xt = pad.T  # (2, NPAD)
    return np.ascontiguousarray(
        np.concatenate([xt[:, :HALF], xt[:, HALF:]], axis=0))


def _unpack_output(op):
    """(128, OPW) packed -> mu (4, NC), lam (3, NC)."""
    mu = np.empty((4, NPAD), np.float32)
    lam = np.empty((3, NPAD), np.float32)
    inv = [0, 2, 1, 3]  # packed col order [mu0, mu2, mu1, mu3]: row e at col inv[e]
    cs = 0
    os_ = 0
    for C in CHUNKS:
        G = C // 128
        blk = op[:, os_:os_ + 14 * G].reshape(128, G, 2, 7)
        v = blk.transpose(3, 2, 1, 0).reshape(7, 2, G * 128)
        for h in range(2):
            dst = slice(h * HALF + cs, h * HALF + cs + C)
            mu[:, dst] = v[inv, h]
            lam[:, dst] = v[4:7, h]
        cs += C
        os_ += 14 * G
    return mu[:, :NC], lam[:, :NC]


_CACHED_NC = None


def _build_nc():
    global _CACHED_NC
    if _CACHED_NC is not None:
        return _CACHED_NC
    nc = bacc.Bacc("TRN2", target_bir_lowering=False, debug=False,
                   enable_asserts=False, num_devices=M)
    xp = nc.dram_tensor("xp", [4, HALF], FM, kind="ExternalInput").ap()
    w1p = nc.dram_tensor("w1p", [4, 128], FM, kind="ExternalInput").ap()
    w2p = nc.dram_tensor("w2p", [128, 128], FM, kind="ExternalInput").ap()
    w3p = nc.dram_tensor("w3p", [128, 128], FM, kind="ExternalInput").ap()
    whp = nc.dram_tensor("whp", [128, 14], FM, kind="ExternalInput").ap()
    b123 = nc.dram_tensor("b123", [128, 3], FP, kind="ExternalInput").ap()
    bh = nc.dram_tensor("bh", [128, 14], FP, kind="ExternalInput").ap()
    op = nc.dram_tensor("op", [128, OPW], FP, kind="ExternalOutput").ap()
    with tile.TileContext(nc) as tc:
        with ExitStack() as ctx:
            body(ctx, tc, xp, w1p, w2p, w3p, whp, b123, bh, op)
    nc.compile()
    _CACHED_NC = nc
    return nc


def kernel(point_cloud, W1, b1, W2, b2, W3, b3, Wmu, bmu, Wlam, blam,
           _want_results=False, **run_kwargs):
    consts = _prep_consts(W1, b1, W2, b2, W3, b3, Wmu, bmu, Wlam, blam)
    consts = {k: np.ascontiguousarray(v, np.float32)
              for k, v in consts.items()}
    pc = np.asarray(point_cloud, np.float32)
    in_maps = []
    for c in range(M):
        m = dict(consts)
        m["xp"] = _pack_input(pc[c * NC:(c + 1) * NC])
        in_maps.append(m)
    nc = _build_nc()
    res = run_bass_kernel_spmd(nc, in_maps, core_ids=list(range(M)),
                               **run_kwargs)
    mu = np.empty((4, N), np.float32)
    lam = np.empty((3, N), np.float32)
    for c in range(M):
        mu_c, lam_c = _unpack_output(res.results[c]["op"])
        mu[:, c * NC:(c + 1) * NC] = mu_c
        lam[:, c * NC:(c + 1) * NC] = lam_c
    if _want_results:
        return (mu, lam), res
    return mu, lam
